# revision 1
# baseline (speedup 1.0000x reference)
"""CTLSTM (continuous-time LSTM, state re-init variant) Trainium2 kernel.

Key insight: the reference re-initializes h/c/c_bar to zero every timestep, so
the 7H gate pre-activations depend ONLY on the event type (1001 distinct
embedding rows), not on the token. We therefore:

  Phase 1 (per core, tensor-parallel over H):
    G = emb_aug @ W_aug.T for this core's 128-wide H slice of the 5 *used*
    gates (i, z, o, ibar, d; f and fbar are dead in the reference), bias
    folded in via an augmented contraction row. Then build a merged per-type
    table [1024 types x 512] = [C | CB | GO | GD] where
      C = sigmoid(gi)*tanh(gz), CB = sigmoid(gib)*tanh(gz),
      GO = sigmoid(go),         GD = softplus(gd).

  Phase 2 (per core, all 16384 tokens):
    dma_gather table rows by event id -> [128 tokens x 512] tiles,
    c_d = CB + (C-CB)*exp(-GD*dur), h_d = GO*tanh(c_d),
    write 5 outputs; C/CB/GO/GD outputs are the gathered slices themselves.

Sharding: every core runs the identical program on ALL tokens but owns H
columns [128k, 128k+128). Host pre-transposes emb/W slices (input
marshaling), and re-interleaves the 8 per-core outputs at the end.
"""

import os

import numpy as np

HIDDEN = 1024
TYPES = 1001
TPAD = 1024          # padded type count (8 m-tiles of 128)
B = 32
T = 512
NTOK = B * T         # 16384
NCORES = 8
KAUG = 1152          # 9 k-tiles of 128 (1024 contraction + 1 bias row + pad)
NGATES = 5           # i, z, o, ibar, d  (f, fbar unused by the reference)
GATE_ROWS = (0, 2, 3, 4, 6)  # row-group index of each used gate in W_rec/b_rec
NCOLS = NGATES * 128  # 640 gate columns per core
WAVE = 1024          # tokens per phase-2 wave
NWAVES = NTOK // WAVE
WCH = WAVE // 128    # chunks per wave (16)

# Set KERNEL_TRACE=1 (or BASS_TRACE=1) to capture an NTFF profile; the
# BassKernelResults of the last run is stashed in LAST_RESULTS.
LAST_RESULTS = None
_CACHED_NC = None
# fp32 tables: rel err ~1e-4, ~13.4us/wave. bf16 tables: rel err ~1e-3,
# ~10.5us/wave (gather reads halve). Flip with KERNEL_F32_TABLES=1.
F32_TABLES = os.environ.get("KERNEL_F32_TABLES", "1") == "1"
# bf16 phase-1 matmul (KERNEL_MM_BF16=1): ~7-10us faster (halved weight-load
# cost + input DMA) but rel err ~1.8e-3 vs 9.3e-5 with float32r. Default f32r.
MM_BF16 = os.environ.get("KERNEL_MM_BF16", "0") == "1"


def _build_nc():
    import concourse.mybir as mybir
    from concourse import bacc
    from concourse.tile import TileContext

    dt = mybir.dt
    AF = mybir.ActivationFunctionType
    f32 = dt.float32
    f32r = dt.float32r
    tdt = f32 if F32_TABLES else dt.bfloat16
    mdt = dt.bfloat16 if MM_BF16 else f32r

    nc = bacc.Bacc("TRN2", target_bir_lowering=False, debug=False)

    et_d = nc.dram_tensor("et", [KAUG, TPAD], mdt, kind="ExternalInput")
    wt_d = nc.dram_tensor("wt", [KAUG, NCOLS], mdt, kind="ExternalInput")
    idx_d = nc.dram_tensor("idx", [128, NTOK // 16], dt.int16, kind="ExternalInput")
    dur_d = nc.dram_tensor("durneg", [128, NTOK // 128], f32, kind="ExternalInput")
    # Output layout: [s, p, chunk, e] where token = chunk*128 + p = t*32 + b
    # and e is the core-local H offset. Host re-interleaves.
    out_d = nc.dram_tensor("out", [5, 128, NTOK // 128, 128], f32, kind="ExternalOutput")

    with TileContext(nc) as tc:
        with (
            tc.tile_pool(name="const", bufs=1) as cpool,
            tc.tile_pool(name="dram", bufs=1, space="DRAM") as dpool,
            tc.tile_pool(name="psum", bufs=2, space="PSUM") as ppool,
            tc.tile_pool(name="eplg", bufs=2) as epool,
            tc.tile_pool(name="wave", bufs=3) as wpool,
        ):
            # ---- constant loads -------------------------------------------------
            et_sb = cpool.tile([128, KAUG // 128, TPAD], mdt, tag="et")
            et_r = et_d[:].rearrange("(t p) m -> p t m", p=128)
            wt_sb = cpool.tile([128, KAUG // 128, NCOLS], mdt, tag="wt")
            wt_r = wt_d[:].rearrange("(t p) n -> p t n", p=128)
            for kt in range(KAUG // 128):
                nc.sync.dma_start(out=wt_sb[:, kt, :], in_=wt_r[:, kt, :])
                nc.sync.dma_start(out=et_sb[:, kt, :], in_=et_r[:, kt, :])
            idx_sb = cpool.tile([128, NTOK // 16], dt.int16, tag="idx")
            nc.sync.dma_start(out=idx_sb[:], in_=idx_d[:])
            dur_sb = cpool.tile([128, NTOK // 128], f32, tag="dur")
            nc.sync.dma_start(out=dur_sb[:], in_=dur_d[:])

            # merged per-type table [C | CB | GO | GD]
            table = dpool.tile([TPAD, 512], tdt, tag="table")

            # ---- phase 1: gate tables ------------------------------------------
            # gd pre-activations staged here so Exp/Ln (softplus = Ln(1+Exp))
            # runs once after the m-loop — avoids ACT-table thrashing.
            gd_all = cpool.tile([128, TPAD // 128, 128], f32, tag="gdall")
            gd_exp = cpool.tile([128, TPAD // 128, 128], f32, tag="gdexp")
            gd_out = cpool.tile([128, TPAD // 128, 128], tdt, tag="gdout")
            for m in range(TPAD // 128):
                psA = ppool.tile([128, 384], f32, tag="psA")  # gi|gz|go
                psB = ppool.tile([128, 256], f32, tag="psB")  # gib|gd
                for kt in range(KAUG // 128):
                    lhs = et_sb[:, kt, m * 128:(m + 1) * 128]
                    first = kt == 0
                    last = kt == KAUG // 128 - 1
                    nc.tensor.matmul(psA[:, :], lhs,
                                     wt_sb[:, kt, 0:384],
                                     start=first, stop=last)
                    nc.tensor.matmul(psB[:, :], lhs,
                                     wt_sb[:, kt, 384:640],
                                     start=first, stop=last)
                merged = epool.tile([128, 384], tdt, tag="merged")
                gi = epool.tile([128, 128], f32, tag="gi")
                gz = epool.tile([128, 128], f32, tag="gz")
                gib = epool.tile([128, 128], f32, tag="gib")
                nc.scalar.activation(out=gi[:], in_=psA[:, 0:128], func=AF.Sigmoid)
                nc.scalar.activation(out=gz[:], in_=psA[:, 128:256], func=AF.Tanh)
                nc.scalar.activation(out=merged[:, 256:384], in_=psA[:, 256:384], func=AF.Sigmoid)
                nc.scalar.activation(out=gib[:], in_=psB[:, 0:128], func=AF.Sigmoid)
                nc.vector.tensor_copy(out=gd_all[:, m, :], in_=psB[:, 128:256])
                nc.vector.tensor_mul(out=merged[:, 0:128], in0=gi[:], in1=gz[:])
                nc.vector.tensor_mul(out=merged[:, 128:256], in0=gib[:], in1=gz[:])
                nc.sync.dma_start(out=table[m * 128:(m + 1) * 128, 0:384], in_=merged[:])
                # softplus(gd) = Ln(1 + Exp(gd)); flush in two halves so most
                # of the GD table slice lands while later m-tiles still matmul
                if m in (3, TPAD // 128 - 1):
                    lo, hi = (0, 4) if m == 3 else (4, TPAD // 128)
                    nc.scalar.activation(out=gd_exp[:, lo:hi, :],
                                         in_=gd_all[:, lo:hi, :], func=AF.Exp)
                    nc.scalar.activation(out=gd_out[:, lo:hi, :],
                                         in_=gd_exp[:, lo:hi, :], func=AF.Ln, bias=1.0)
                    nc.sync.dma_start(
                        out=table[lo * 128:hi * 128, 384:512].rearrange(
                            "(m p) e -> p m e", p=128),
                        in_=gd_out[:, lo:hi, :],
                    )

            # ---- phase 2: gather + pointwise -----------------------------------
            for w in range(NWAVES):
                ws = slice(w * WCH, (w + 1) * WCH)
                g = wpool.tile([128, WCH, 512], tdt, tag="g")
                nc.gpsimd.dma_gather(
                    g[:],
                    table[:],
                    idx_sb[:, w * (WAVE // 16):(w + 1) * (WAVE // 16)],
                    WAVE,
                    WAVE,
                    512,
                    single_packet=False,
                )
                if F32_TABLES:
                    for si, c0 in ((1, 0), (2, 128), (3, 256), (4, 384)):
                        nc.sync.dma_start(
                            out=out_d[si, :, ws, :], in_=g[:, :, c0:c0 + 128])
                else:
                    # bf16 -> f32 casts split across DVE and ACT, then HWDGE out
                    gc = wpool.tile([128, WCH, 512], f32, tag="gc")
                    nc.vector.tensor_copy(out=gc[:, :, 0:128], in_=g[:, :, 0:128])
                    nc.vector.tensor_copy(out=gc[:, :, 128:256], in_=g[:, :, 128:256])
                    nc.scalar.copy(out=gc[:, :, 256:384], in_=g[:, :, 256:384])
                    nc.scalar.copy(out=gc[:, :, 384:512], in_=g[:, :, 384:512])
                    for si, c0 in ((1, 0), (2, 128), (3, 256), (4, 384)):
                        nc.sync.dma_start(
                            out=out_d[si, :, ws, :], in_=gc[:, :, c0:c0 + 128])
                te = wpool.tile([128, WCH, 128], f32, tag="te")
                for c in range(WCH):
                    wc = w * WCH + c
                    nc.scalar.activation(
                        out=te[:, c, :], in_=g[:, c, 384:512], func=AF.Exp,
                        scale=dur_sb[:, wc:wc + 1],
                    )
                td = wpool.tile([128, WCH, 128], f32, tag="td")
                nc.vector.tensor_sub(td[:], g[:, :, 0:128], g[:, :, 128:256])
                nc.vector.tensor_mul(td[:], td[:], te[:])
                nc.vector.tensor_add(td[:], td[:], g[:, :, 128:256])
                nc.scalar.activation(out=te[:], in_=td[:], func=AF.Tanh)
                nc.vector.tensor_mul(td[:], te[:], g[:, :, 256:384])
                nc.sync.dma_start(out=out_d[0, :, ws, :], in_=td[:])

    nc.compile()
    return nc


def _ensure_ntff_hook():
    """The agent image's antenv lacks axon_hooks; shim it and register the
    ctypes NTFF profiling hook so trace=True works under axon."""
    import sys
    import types

    try:
        from antenv.axon_hooks import get_axon_ntff_profile_hook  # noqa: F401
        return
    except ImportError:
        pass
    try:
        import antenv
    except ImportError:
        return
    mod = types.ModuleType("antenv.axon_hooks")
    state = {"hook": None}
    mod.set_axon_ntff_profile_hook = lambda h: state.__setitem__("hook", h)
    mod.get_axon_ntff_profile_hook = lambda: state["hook"]
    sys.modules["antenv.axon_hooks"] = mod
    antenv.axon_hooks = mod
    try:
        from trn_agent_boot.trn_boot import _ntff_profile_via_ctypes

        hook = _ntff_profile_via_ctypes("/opt/axon/libaxon_pjrt.so")
        if hook is not None:
            mod.set_axon_ntff_profile_hook(hook)
    except Exception:
        pass


def kernel(event_seqs, duration_seqs, emb_table, W_rec, b_rec):
    global LAST_RESULTS, _CACHED_NC
    from concourse.bass_utils import run_bass_kernel_spmd

    ev = np.asarray(event_seqs)
    dur = np.asarray(duration_seqs, dtype=np.float32)
    emb = np.asarray(emb_table, dtype=np.float32)
    W = np.asarray(W_rec, dtype=np.float32)
    b = np.asarray(b_rec, dtype=np.float32)

    # ---- host-side input marshaling (sharding) -----------------------------
    et = np.zeros((KAUG, TPAD), np.float32)
    et[:HIDDEN, :TYPES] = emb.T
    et[HIDDEN, :] = 1.0  # bias row

    ev_tok = ev.T.reshape(-1).astype(np.int16)          # token t*32+b -> type
    # idx i at [i%16, i//16], replicated across the 8 GPSIMD core stripes
    idx = np.tile(ev_tok.reshape(-1, 16).T, (8, 1)).astype(np.int16)

    dur_tok = dur.T.reshape(-1)
    durneg = np.ascontiguousarray((-dur_tok).reshape(-1, 128).T)  # [128, chunks]

    if MM_BF16:
        import ml_dtypes
        et = et.astype(ml_dtypes.bfloat16)
    in_maps = []
    for k in range(NCORES):
        h0 = 128 * k
        wt = np.zeros((KAUG, NCOLS), np.float32)
        for g5, g7 in enumerate(GATE_ROWS):
            rows = slice(g7 * HIDDEN + h0, g7 * HIDDEN + h0 + 128)
            wt[:HIDDEN, g5 * 128:(g5 + 1) * 128] = W[rows, :HIDDEN].T
            wt[HIDDEN, g5 * 128:(g5 + 1) * 128] = b[rows]
        if MM_BF16:
            import ml_dtypes
            wt = wt.astype(ml_dtypes.bfloat16)
        in_maps.append({"et": et, "wt": wt, "idx": idx, "durneg": durneg})

    if _CACHED_NC is None:
        _CACHED_NC = _build_nc()
    nc = _CACHED_NC

    trace = os.environ.get("KERNEL_TRACE", "") not in ("", "0")
    if trace:
        _ensure_ntff_hook()
    res = run_bass_kernel_spmd(nc, in_maps, list(range(NCORES)), trace=trace)
    LAST_RESULTS = res

    # ---- host-side output assembly ----------------------------------------
    full = np.empty((5, T, B, HIDDEN), np.float32)
    flat = full.reshape(5, NTOK, HIDDEN)
    for k in range(NCORES):
        o = res.results[k]["out"]  # [5, 128, chunks, 128]
        flat[:, :, 128 * k:128 * (k + 1)] = o.transpose(0, 2, 1, 3).reshape(5, NTOK, 128)
    return full

